# revision 9
# baseline (speedup 1.0000x reference)
"""MixtureOfSoftMaxACF Trainium2 kernel (fp16, ACT-bound pipeline).

Per-core (data-parallel over BS=8 across 8 cores, batch b per core):
  qt[b] memory reinterpreted as QQ[2, 2048, 64] (contiguous halves), same kt.
  For m in {0,1}:  S_m = QQ[m] @ KK[m].T / sqrt(128);  P_m = softmax(S_m, axis=-1)
  out[b] = (p0 * P_0 + p1 * P_1) @ vt[b]
  p: mixture prior (softmax over batch axis) -> computed on host, passed per-core.

Device pipeline per core, per (qh, m):
  - Scores: S^T [128 keys, 1024 q] = lhsT(K^T chunk [64,128]) @ rhs(Q^T slab),
    fp16 inputs, two 512-wide matmuls (PSUM bank limit).
  - exp on ScalarE from PSUM -> E fp16 SBUF (scale=1/sqrt(128)); ScalarE is the
    bottleneck engine, everything else is arranged to hide behind it.
  - AV (V-stationary) deferred one chunk so PE never waits on the current exp.
  - Softmax denominator OFF the PE: DVE pairwise adds E(2j)+E(2j+1) in-loop
    (fp16 2x mode), small tree post-loop, then one (ones/p_m)-stationary matmul
    replicates D/p_m across partitions; reciprocal gives p_m/D directly.
  - Normalize in the [dv, q] domain, sum mixtures, PE-transpose to [q, dv],
    DVE copy, DMA out. Cross-boundary work overlaps the next chunk loop.
"""

import math
from contextlib import ExitStack

import numpy as np

import concourse.bass as bass
import concourse.bacc as bacc
import concourse.mybir as mybir
import concourse.tile as tile
from concourse.bass_utils import run_bass_kernel_spmd
from concourse.masks import make_identity

BS = 8
N = 2048          # queries
NK = 2048         # keys
DK = 128
M = 2
D = DK // M       # 64
DV = 128
TEMP = math.sqrt(DK)
NCH = NK // 128   # 16 key chunks
QH = 2            # query halves
QHN = N // QH     # 1024

F32 = mybir.dt.float32
F16 = mybir.dt.float16

_NC = None
LAST_RESULT = None  # BassKernelResults of last run (test.py reads this)


def _build():
    nc = bacc.Bacc(None)
    qt_d = nc.declare_dram_parameter("qt_b", [N, DK], F32, isOutput=False)
    kt_d = nc.declare_dram_parameter("kt_b", [NK, DK], F32, isOutput=False)
    vt_d = nc.declare_dram_parameter("vt_b", [NK, DK], F32, isOutput=False)
    pr_d = nc.declare_dram_parameter("pr_b", [1, M], F32, isOutput=False)
    out_d = nc.declare_dram_parameter("out_b", [N, DK], F32, isOutput=True)

    with ExitStack() as ctx:
        tc = ctx.enter_context(tile.TileContext(nc))
        const = ctx.enter_context(tc.tile_pool(name="const", bufs=1))
        sbig = ctx.enter_context(tc.tile_pool(name="sbig", bufs=1))
        epool = ctx.enter_context(tc.tile_pool(name="epool", bufs=4))
        eppool = ctx.enter_context(tc.tile_pool(name="eppool", bufs=2))
        dpool = ctx.enter_context(tc.tile_pool(name="dpool", bufs=2))
        npool = ctx.enter_context(tc.tile_pool(name="npool", bufs=2))
        ps_s = ctx.enter_context(tc.tile_pool(name="ps_s", bufs=2, space="PSUM"))
        ps_acc = ctx.enter_context(tc.tile_pool(name="ps_acc", bufs=2, space="PSUM"))

        # ---- constants ----
        ident_f = const.tile([128, 128], F32)
        make_identity(nc, ident_f)
        ones_f = const.tile([128, 128], F32)
        nc.vector.memset(ones_f, 1.0)
        pr_sb = const.tile([128, M], F32)
        nc.sync.dma_start(
            out=pr_sb,
            in_=bass.AP(tensor=pr_d, offset=0, ap=[[0, 128], [1, M]]),
        )
        # 1/p_m broadcast on all partitions, then ones/p_m weight tiles (fp16)
        pr_rec = const.tile([128, M], F32)
        nc.vector.reciprocal(pr_rec, pr_sb)
        ones_p = []
        for m in range(M):
            t = const.tile([128, 128], F16, tag=f"ones_p{m}")
            nc.vector.tensor_scalar_mul(t, ones_f, pr_rec[:, m:m + 1])
            ones_p.append(t)

        # ---- PE warm-up: ~40 dependency-free matmuls back-to-back while the
        # input DMAs run. HAM un-throttles the PE clock (1.2->2.4 GHz) only
        # after ~3.4us of *sustained* PE activity; the main loop's bursts are
        # too short to ever trigger it on their own.
        warm_w = const.tile([128, 128], F16, tag="warm_w")
        nc.vector.tensor_copy(warm_w, ones_f)
        warm_w2 = const.tile([128, 128], F16, tag="warm_w2")
        nc.vector.tensor_copy(warm_w2, ones_f)
        warm_r = const.tile([128, 512], F16, tag="warm_r")
        nc.vector.memset(warm_r, 0.5)
        warm_ps = ps_acc.tile([128, QHN], F32, tag="outT")
        # block A: same weights, same bank
        for _ in range(40):
            nc.tensor.matmul(warm_ps[:, 0:512], lhsT=warm_w, rhs=warm_r,
                             start=True, stop=True)
        # block B: same weights, alternating banks
        for i in range(24):
            sl = slice(0, 512) if i % 2 == 0 else slice(512, 1024)
            nc.tensor.matmul(warm_ps[:, sl], lhsT=warm_w, rhs=warm_r,
                             start=True, stop=True)
        # block C: alternating weights, same bank
        for i in range(24):
            nc.tensor.matmul(warm_ps[:, 0:512],
                             lhsT=(warm_w if i % 2 == 0 else warm_w2),
                             rhs=warm_r, start=True, stop=True)
        # block D: alternating weights and banks
        for i in range(24):
            sl = slice(0, 512) if i % 2 == 0 else slice(512, 1024)
            nc.tensor.matmul(warm_ps[:, sl],
                             lhsT=(warm_w if i % 2 == 0 else warm_w2),
                             rhs=warm_r, start=True, stop=True)

        # ---- input staging: [128, 16, (m,d)] so stage[:, c, :] is a [128, 128]
        # block whose transpose has mixture m's d-rows at partitions m*64..m*64+63.
        # stage[p, c, m*64+d] = flat[m*131072 + (c*128+p)*64 + d]
        stages = []
        for src in (qt_d, kt_d):
            t = sbig.tile([128, NCH, DK], F32, tag=f"stage{len(stages)}")
            for m in range(M):
                nc.sync.dma_start(
                    out=t[:, :, m * D:(m + 1) * D],
                    in_=bass.AP(
                        tensor=src, offset=m * N * D,
                        ap=[[D, 128], [128 * D, NCH], [1, D]],
                    ),
                )
            stages.append(t)

        # V: [128, 16, 128]  (p, c, dv) <- vt[c*128+p, dv]; convert to fp16
        v_st = sbig.tile([128, NCH, DV], F32)
        nc.sync.dma_start(
            out=v_st,
            in_=bass.AP(tensor=vt_d, offset=0,
                        ap=[[DK, 128], [128 * DK, NCH], [1, DV]]),
        )
        v_sb = sbig.tile([128, NCH, DV], F16)
        nc.vector.tensor_copy(v_sb, v_st)

        # ---- phase 1: QT/KT [128, 2048] fp16 (rows m*64+d), via PE transpose ----
        qt_t = sbig.tile([128, N], F16)
        kt_t = sbig.tile([128, NK], F16)
        for stage, dst in ((stages[0], qt_t), (stages[1], kt_t)):
            for c in range(NCH):
                tp = ps_s.tile([128, QHN], F32, tag="s")
                nc.tensor.transpose(tp[:, 0:128], stage[:, c, :], ident_f)
                nc.vector.tensor_copy(dst[:, c * 128:(c + 1) * 128], tp[:, 0:128])

        # ---- phase 2+3: attention ----
        # Normalization of segment i is deferred into segment i+1's chunk loop
        # (tree-adds early, Drep matmul late) so the PE queue never blocks on
        # DVE work at a boundary: PE goes straight from AV(15) to next S(0),
        # which also keeps the HAM clock-gate warm.
        scale = 1.0 / TEMP
        stage1 = {}  # c -> closure, emitted during the chunk loop
        stage2 = []  # closures, emitted after the full sweep

        def emit_stage1(c):
            for fn in stage1.pop(c, ()):  # noqa: B909
                fn()

        otn = {}
        daccs = {}
        segs = [(qh, m) for qh in range(QH) for m in range(M)]
        for qh, m in segs:
            outT = ps_acc.tile([128, QHN], F32, tag="outT")
            ep = eppool.tile([128, NCH // 2, QHN], F16, tag="ep")
            es = []
            for c in range(NCH):
                s = ps_s.tile([128, QHN], F32, tag="s")
                for hf in range(2):
                    nc.tensor.matmul(
                        s[:, hf * 512:(hf + 1) * 512],
                        lhsT=kt_t[m * D:(m + 1) * D, c * 128:(c + 1) * 128],
                        rhs=qt_t[m * D:(m + 1) * D,
                                 qh * QHN + hf * 512: qh * QHN + (hf + 1) * 512],
                        start=True, stop=True,
                    )
                # deferred AV: consume E(c-1) while exp(c) runs
                if c >= 1:
                    Ep = es[c - 1]
                    for hf in range(2):
                        sl = slice(hf * 512, (hf + 1) * 512)
                        nc.tensor.matmul(outT[:, sl], lhsT=v_sb[:, c - 1, :],
                                         rhs=Ep[:, sl],
                                         start=(c == 1), stop=False)
                E = epool.tile([128, QHN], F16, tag="E")
                nc.scalar.activation(E, s, mybir.ActivationFunctionType.Exp,
                                     scale=scale)
                es.append(E)
                if c % 2 == 1:
                    nc.vector.tensor_add(ep[:, c // 2, :], es[c - 1], es[c])
                emit_stage1(c)
            # tail AV for chunk 15
            for hf in range(2):
                sl = slice(hf * 512, (hf + 1) * 512)
                nc.tensor.matmul(outT[:, sl], lhsT=v_sb[:, NCH - 1, :],
                                 rhs=es[NCH - 1][:, sl],
                                 start=False, stop=True)

            def tree(qh=qh, m=m, ep=ep):
                # denominator tree: 8 pairs -> 4 -> 2 -> 1 (fp16), on DVE
                t4 = dpool.tile([128, 4, QHN], F16, tag="t4")
                nc.vector.tensor_add(t4, ep[:, 0:4, :], ep[:, 4:8, :])
                t2 = dpool.tile([128, 2, QHN], F16, tag="t2")
                nc.vector.tensor_add(t2, t4[:, 0:2, :], t4[:, 2:4, :])
                dacc = dpool.tile([128, QHN], F16, tag=f"dacc{m}")
                nc.vector.tensor_add(dacc, t2[:, 0, :], t2[:, 1, :])
                daccs[(qh, m)] = dacc

            def norm(qh=qh, m=m, outT=outT):
                # replicate D/p_m across partitions with a ones/p_m matmul
                dacc = daccs.pop((qh, m))
                Drep = ps_s.tile([128, QHN], F32, tag="s")
                for hf in range(2):
                    sl = slice(hf * 512, (hf + 1) * 512)
                    nc.tensor.matmul(Drep[:, sl], lhsT=ones_p[m], rhs=dacc[:, sl],
                                     start=True, stop=True)
                drec = npool.tile([128, QHN], F32, tag=f"drec{m}")
                nc.vector.reciprocal_approx_fast(drec, Drep)
                t = npool.tile([128, QHN], F32, tag=f"otn{qh}{m}")
                nc.vector.tensor_mul(t, outT, drec)
                otn[(qh, m)] = t

            def finish(qh=qh):
                rT2 = npool.tile([128, QHN], F32, tag="rT2")
                nc.vector.tensor_add(rT2, otn[(qh, 0)], otn[(qh, 1)])
                res_ps = ps_s.tile([128, QHN], F32, tag="s")
                for t in range(QHN // 128):
                    nc.tensor.transpose(res_ps[:, t * 128:(t + 1) * 128],
                                        rT2[:, t * 128:(t + 1) * 128], ident_f)
                res_sb = npool.tile([128, QHN], F32, tag="res")
                nc.vector.tensor_copy(res_sb, res_ps)
                nc.sync.dma_start(
                    out=bass.AP(tensor=out_d, offset=qh * QHN * DK,
                                ap=[[DK, 128], [128 * DK, QHN // 128], [1, DV]]),
                    in_=res_sb.rearrange("p (t d) -> p t d", d=DV),
                )

            stage1.setdefault(2, []).append(tree)
            stage1.setdefault(8, []).append(norm)
            if m == M - 1:
                stage1.setdefault(12, []).append(finish)
            stage2.extend([tree, norm] if (qh, m) == segs[-1] else [])
            if (qh, m) == segs[-1]:
                stage2.append(finish)

        # last segment's deferred work never found a next loop; emit it now.
        stage1.clear()
        for fn in stage2:
            fn()
    return nc


def _get_nc():
    global _NC
    if _NC is None:
        _NC = _build()
        _NC.finalize()  # Bacc.compile(): event sems, reg alloc, wait legalization
    return _NC


def _prior(qt, kernel):
    bar_qt = qt.astype(np.float32).mean(axis=1)          # (BS, dk)
    logits = kernel.astype(np.float32) @ bar_qt.T        # (m, BS)
    z = logits - logits.max(axis=1, keepdims=True)
    ez = np.exp(z)
    pm = ez / ez.sum(axis=1, keepdims=True)              # softmax over batch axis
    return pm.reshape(-1)


def kernel(qt, kt, vt, kernel):
    global LAST_RESULT
    import os
    nc = _get_nc()
    prior_flat = _prior(qt, kernel)
    in_maps = []
    for b in range(BS):
        pr = np.array([[prior_flat[2 * b], prior_flat[2 * b + 1]]], dtype=np.float32)
        in_maps.append({
            "qt_b": np.ascontiguousarray(qt[b], dtype=np.float32),
            "kt_b": np.ascontiguousarray(kt[b], dtype=np.float32),
            "vt_b": np.ascontiguousarray(vt[b], dtype=np.float32),
            "pr_b": pr,
        })
    trace = bool(int(os.environ.get("KERNEL_TRACE", "0")))
    res = run_bass_kernel_spmd(nc, in_maps, list(range(BS)), trace=trace)
    LAST_RESULT = res
    out = np.stack([np.asarray(res.results[b]["out_b"]).reshape(N, DK) for b in range(BS)])
    return out.astype(np.float32)


# revision 11
# speedup vs baseline: 1.1863x; 1.1863x over previous
"""MixtureOfSoftMaxACF Trainium2 kernel (fp16, ACT-bound pipeline).

Per-core (data-parallel over BS=8 across 8 cores, batch b per core):
  qt[b] memory reinterpreted as QQ[2, 2048, 64] (contiguous halves), same kt.
  For m in {0,1}:  S_m = QQ[m] @ KK[m].T / sqrt(128);  P_m = softmax(S_m, axis=-1)
  out[b] = (p0 * P_0 + p1 * P_1) @ vt[b]
  p: mixture prior (softmax over batch axis) -> computed on host, passed per-core.

Device pipeline per core, per (qh, m):
  - Scores: S^T [128 keys, 1024 q] = lhsT(K^T chunk [64,128]) @ rhs(Q^T slab),
    fp16 inputs, two 512-wide matmuls (PSUM bank limit).
  - exp on ScalarE from PSUM -> E fp16 SBUF (scale=1/sqrt(128)); ScalarE is the
    bottleneck engine, everything else is arranged to hide behind it.
  - AV (V-stationary) deferred one chunk so PE never waits on the current exp.
  - Softmax denominator OFF the PE: DVE pairwise adds E(2j)+E(2j+1) in-loop
    (fp16 2x mode), small tree post-loop, then one (ones/p_m)-stationary matmul
    replicates D/p_m across partitions; reciprocal gives p_m/D directly.
  - Normalize in the [dv, q] domain, sum mixtures, PE-transpose to [q, dv],
    DVE copy, DMA out. Cross-boundary work overlaps the next chunk loop.
"""

import math
from contextlib import ExitStack

import numpy as np

import concourse.bass as bass
import concourse.bacc as bacc
import concourse.mybir as mybir
import concourse.tile as tile
from concourse.bass_utils import run_bass_kernel_spmd
from concourse.masks import make_identity

BS = 8
N = 2048          # queries
NK = 2048         # keys
DK = 128
M = 2
D = DK // M       # 64
DV = 128
TEMP = math.sqrt(DK)
NCH = NK // 128   # 16 key chunks
QH = 2            # query halves
QHN = N // QH     # 1024

F32 = mybir.dt.float32
F16 = mybir.dt.float16

_NC = None
LAST_RESULT = None  # BassKernelResults of last run (test.py reads this)


def _build():
    nc = bacc.Bacc(None)
    qt_d = nc.declare_dram_parameter("qt_b", [N, DK], F32, isOutput=False)
    kt_d = nc.declare_dram_parameter("kt_b", [NK, DK], F32, isOutput=False)
    vt_d = nc.declare_dram_parameter("vt_b", [NK, DK], F32, isOutput=False)
    pr_d = nc.declare_dram_parameter("pr_b", [1, M], F32, isOutput=False)
    out_d = nc.declare_dram_parameter("out_b", [N, DK], F32, isOutput=True)

    with ExitStack() as ctx:
        tc = ctx.enter_context(tile.TileContext(nc))
        const = ctx.enter_context(tc.tile_pool(name="const", bufs=1))
        sbig = ctx.enter_context(tc.tile_pool(name="sbig", bufs=1))
        epool = ctx.enter_context(tc.tile_pool(name="epool", bufs=5))
        eppool = ctx.enter_context(tc.tile_pool(name="eppool", bufs=2))
        dpool = ctx.enter_context(tc.tile_pool(name="dpool", bufs=2))
        npool = ctx.enter_context(tc.tile_pool(name="npool", bufs=2))
        opool = ctx.enter_context(tc.tile_pool(name="opool", bufs=1))
        ps_s = ctx.enter_context(tc.tile_pool(name="ps_s", bufs=3, space="PSUM"))
        ps_acc = ctx.enter_context(tc.tile_pool(name="ps_acc", bufs=1, space="PSUM"))

        # ---- constants ----
        ident_f = const.tile([128, 128], F32)
        make_identity(nc, ident_f)
        ones_f = const.tile([128, 128], F32)
        nc.vector.memset(ones_f, 1.0)
        pr_sb = const.tile([128, M], F32)
        nc.sync.dma_start(
            out=pr_sb,
            in_=bass.AP(tensor=pr_d, offset=0, ap=[[0, 128], [1, M]]),
        )
        # 1/p_m broadcast on all partitions, then ones/p_m weight tiles (fp16)
        pr_rec = const.tile([128, M], F32)
        nc.vector.reciprocal(pr_rec, pr_sb)
        ones_p = []
        for m in range(M):
            t = const.tile([128, 128], F16, tag=f"ones_p{m}")
            nc.vector.tensor_scalar_mul(t, ones_f, pr_rec[:, m:m + 1])
            ones_p.append(t)

        # ---- PE warm-up: ~40 dependency-free matmuls back-to-back while the
        # input DMAs run. HAM un-throttles the PE clock (1.2->2.4 GHz) only
        # after ~3.4us of *sustained* PE activity; the main loop's bursts are
        # too short to ever trigger it on their own.
        warm_w = const.tile([128, 128], F16, tag="warm_w")
        nc.vector.tensor_copy(warm_w, ones_f)
        warm_r = const.tile([128, 512], F16, tag="warm_r")
        nc.vector.memset(warm_r, 0.5)
        warm_ps = ps_acc.tile([128, QHN], F32, tag="outT")
        for _ in range(40):
            nc.tensor.matmul(warm_ps[:, 0:512], lhsT=warm_w, rhs=warm_r,
                             start=True, stop=True)

        # ---- input staging: [128, 16, (m,d)] so stage[:, c, :] is a [128, 128]
        # block whose transpose has mixture m's d-rows at partitions m*64..m*64+63.
        # stage[p, c, m*64+d] = flat[m*131072 + (c*128+p)*64 + d]
        stages = []
        for src in (qt_d, kt_d):
            t = sbig.tile([128, NCH, DK], F32, tag=f"stage{len(stages)}")
            for m in range(M):
                nc.sync.dma_start(
                    out=t[:, :, m * D:(m + 1) * D],
                    in_=bass.AP(
                        tensor=src, offset=m * N * D,
                        ap=[[D, 128], [128 * D, NCH], [1, D]],
                    ),
                )
            stages.append(t)

        # V: [128, 16, 128]  (p, c, dv) <- vt[c*128+p, dv]; convert to fp16
        v_st = sbig.tile([128, NCH, DV], F32)
        nc.sync.dma_start(
            out=v_st,
            in_=bass.AP(tensor=vt_d, offset=0,
                        ap=[[DK, 128], [128 * DK, NCH], [1, DV]]),
        )
        v_sb = sbig.tile([128, NCH, DV], F16)
        nc.vector.tensor_copy(v_sb, v_st)

        # ---- phase 1: QT/KT [128, 2048] fp16 (rows m*64+d), via PE transpose ----
        qt_t = sbig.tile([128, N], F16)
        kt_t = sbig.tile([128, NK], F16)
        for stage, dst in ((stages[0], qt_t), (stages[1], kt_t)):
            for c in range(NCH):
                tp = ps_s.tile([128, QHN], F32, tag="s")
                nc.tensor.transpose(tp[:, 0:128], stage[:, c, :], ident_f)
                nc.vector.tensor_copy(dst[:, c * 128:(c + 1) * 128], tp[:, 0:128])

        # ---- phase 2+3: attention ----
        # Normalization of segment i is deferred into segment i+1's chunk loop
        # (tree-adds early, Drep matmul late) so the PE queue never blocks on
        # DVE work at a boundary: PE goes straight from AV(15) to next S(0),
        # which also keeps the HAM clock-gate warm.
        scale = 1.0 / TEMP
        stage1 = {}  # c -> closure, emitted during the chunk loop
        stage2 = []  # closures, emitted after the full sweep

        def emit_stage1(c):
            for fn in stage1.pop(c, ()):  # noqa: B909
                fn()

        otn = {}
        daccs = {}
        ocs = {}
        drecs = {}
        segs = [(qh, m) for qh in range(QH) for m in range(M)]
        for qh, m in segs:
            outT = ps_acc.tile([128, QHN], F32, tag="outT")
            ep = eppool.tile([128, NCH // 2, QHN], F16, tag="ep")
            es = []
            for c in range(NCH):
                s = ps_s.tile([128, QHN], F32, tag="s")
                for hf in range(2):
                    nc.tensor.matmul(
                        s[:, hf * 512:(hf + 1) * 512],
                        lhsT=kt_t[m * D:(m + 1) * D, c * 128:(c + 1) * 128],
                        rhs=qt_t[m * D:(m + 1) * D,
                                 qh * QHN + hf * 512: qh * QHN + (hf + 1) * 512],
                        start=True, stop=True,
                    )
                # deferred AV: consume E(c-2); exp(c-2) finished a chunk ago,
                # so the PE never waits on the ScalarE here
                if c >= 2:
                    Ep = es[c - 2]
                    for hf in range(2):
                        sl = slice(hf * 512, (hf + 1) * 512)
                        nc.tensor.matmul(outT[:, sl], lhsT=v_sb[:, c - 2, :],
                                         rhs=Ep[:, sl],
                                         start=(c == 2), stop=False)
                E = epool.tile([128, QHN], F16, tag="E")
                nc.scalar.activation(E, s, mybir.ActivationFunctionType.Exp,
                                     scale=scale)
                es.append(E)
                if c % 2 == 1:
                    nc.vector.tensor_add(ep[:, c // 2, :], es[c - 1], es[c])
                emit_stage1(c)
            # tail AV for chunks 14, 15
            for ct in (NCH - 2, NCH - 1):
                for hf in range(2):
                    sl = slice(hf * 512, (hf + 1) * 512)
                    nc.tensor.matmul(outT[:, sl], lhsT=v_sb[:, ct, :],
                                     rhs=es[ct][:, sl],
                                     start=False, stop=(ct == NCH - 1))
            # free the single outT PSUM buffer right away: plain copy to SBUF;
            # the division by D happens later in SBUF during finish()
            oc = opool.tile([128, QHN], F32, tag=f"oc{qh}{m}")
            nc.vector.tensor_copy(oc, outT)
            ocs[(qh, m)] = oc

            def tree(qh=qh, m=m, ep=ep):
                # denominator tree: 8 pairs -> 4 -> 2 -> 1 (fp16), on DVE
                t4 = dpool.tile([128, 4, QHN], F16, tag="t4")
                nc.vector.tensor_add(t4, ep[:, 0:4, :], ep[:, 4:8, :])
                t2 = dpool.tile([128, 2, QHN], F16, tag="t2")
                nc.vector.tensor_add(t2, t4[:, 0:2, :], t4[:, 2:4, :])
                dacc = dpool.tile([128, QHN], F16, tag=f"dacc{m}")
                nc.vector.tensor_add(dacc, t2[:, 0, :], t2[:, 1, :])
                daccs[(qh, m)] = dacc

            def norm(qh=qh, m=m):
                # replicate D/p_m across partitions with a ones/p_m matmul
                dacc = daccs.pop((qh, m))
                Drep = ps_s.tile([128, QHN], F32, tag="s")
                for hf in range(2):
                    sl = slice(hf * 512, (hf + 1) * 512)
                    nc.tensor.matmul(Drep[:, sl], lhsT=ones_p[m], rhs=dacc[:, sl],
                                     start=True, stop=True)
                drec = opool.tile([128, QHN], F32, tag=f"drec{qh}{m}")
                nc.vector.reciprocal_approx_fast(drec, Drep)
                drecs[(qh, m)] = drec

            def finish(qh=qh):
                rA = npool.tile([128, QHN], F32, tag="rA")
                nc.vector.tensor_mul(rA, ocs[(qh, 0)], drecs[(qh, 0)])
                rB = npool.tile([128, QHN], F32, tag="rB")
                nc.vector.tensor_mul(rB, ocs[(qh, 1)], drecs[(qh, 1)])
                rT2 = npool.tile([128, QHN], F32, tag="rT2")
                nc.vector.tensor_add(rT2, rA, rB)
                res_ps = ps_s.tile([128, QHN], F32, tag="s")
                for t in range(QHN // 128):
                    nc.tensor.transpose(res_ps[:, t * 128:(t + 1) * 128],
                                        rT2[:, t * 128:(t + 1) * 128], ident_f)
                res_sb = npool.tile([128, QHN], F32, tag="res")
                nc.vector.tensor_copy(res_sb, res_ps)
                nc.sync.dma_start(
                    out=bass.AP(tensor=out_d, offset=qh * QHN * DK,
                                ap=[[DK, 128], [128 * DK, QHN // 128], [1, DV]]),
                    in_=res_sb.rearrange("p (t d) -> p t d", d=DV),
                )

            stage1.setdefault(2, []).append(tree)
            stage1.setdefault(8, []).append(norm)
            if m == M - 1:
                stage1.setdefault(12, []).append(finish)
            stage2.extend([tree, norm] if (qh, m) == segs[-1] else [])
            if (qh, m) == segs[-1]:
                stage2.append(finish)

        # last segment's deferred work never found a next loop; emit it now.
        stage1.clear()
        for fn in stage2:
            fn()
    return nc


def _get_nc():
    global _NC
    if _NC is None:
        _NC = _build()
        _NC.finalize()  # Bacc.compile(): event sems, reg alloc, wait legalization
    return _NC


def _prior(qt, kernel):
    bar_qt = qt.astype(np.float32).mean(axis=1)          # (BS, dk)
    logits = kernel.astype(np.float32) @ bar_qt.T        # (m, BS)
    z = logits - logits.max(axis=1, keepdims=True)
    ez = np.exp(z)
    pm = ez / ez.sum(axis=1, keepdims=True)              # softmax over batch axis
    return pm.reshape(-1)


def kernel(qt, kt, vt, kernel):
    global LAST_RESULT
    import os
    nc = _get_nc()
    prior_flat = _prior(qt, kernel)
    in_maps = []
    for b in range(BS):
        pr = np.array([[prior_flat[2 * b], prior_flat[2 * b + 1]]], dtype=np.float32)
        in_maps.append({
            "qt_b": np.ascontiguousarray(qt[b], dtype=np.float32),
            "kt_b": np.ascontiguousarray(kt[b], dtype=np.float32),
            "vt_b": np.ascontiguousarray(vt[b], dtype=np.float32),
            "pr_b": pr,
        })
    trace = bool(int(os.environ.get("KERNEL_TRACE", "0")))
    res = run_bass_kernel_spmd(nc, in_maps, list(range(BS)), trace=trace)
    LAST_RESULT = res
    out = np.stack([np.asarray(res.results[b]["out_b"]).reshape(N, DK) for b in range(BS)])
    return out.astype(np.float32)


# revision 13
# speedup vs baseline: 1.4510x; 1.2231x over previous
"""MixtureOfSoftMaxACF Trainium2 kernel (fp16, ACT-bound pipeline).

Per-core (data-parallel over BS=8 across 8 cores, batch b per core):
  qt[b] memory reinterpreted as QQ[2, 2048, 64] (contiguous halves), same kt.
  For m in {0,1}:  S_m = QQ[m] @ KK[m].T / sqrt(128);  P_m = softmax(S_m, axis=-1)
  out[b] = (p0 * P_0 + p1 * P_1) @ vt[b]
  p: mixture prior (softmax over batch axis) -> computed on host, passed per-core.

Device pipeline per core, per (qh, m):
  - Scores: S^T [128 keys, 1024 q] = lhsT(K^T chunk [64,128]) @ rhs(Q^T slab),
    fp16 inputs, two 512-wide matmuls (PSUM bank limit).
  - exp on ScalarE from PSUM -> E fp16 SBUF (scale=1/sqrt(128)); ScalarE is the
    bottleneck engine, everything else is arranged to hide behind it.
  - AV (V-stationary) deferred one chunk so PE never waits on the current exp.
  - Softmax denominator OFF the PE: DVE pairwise adds E(2j)+E(2j+1) in-loop
    (fp16 2x mode), small tree post-loop, then one (ones/p_m)-stationary matmul
    replicates D/p_m across partitions; reciprocal gives p_m/D directly.
  - Normalize in the [dv, q] domain, sum mixtures, PE-transpose to [q, dv],
    DVE copy, DMA out. Cross-boundary work overlaps the next chunk loop.
"""

import math
from contextlib import ExitStack

import numpy as np

import concourse.bass as bass
import concourse.bacc as bacc
import concourse.mybir as mybir
import concourse.tile as tile
from concourse.bass_utils import run_bass_kernel_spmd
from concourse.masks import make_identity

BS = 8
N = 2048          # queries
NK = 2048         # keys
DK = 128
M = 2
D = DK // M       # 64
DV = 128
TEMP = math.sqrt(DK)
NCH = NK // 128   # 16 key chunks
QH = 2            # query halves
QHN = N // QH     # 1024

F32 = mybir.dt.float32
F16 = mybir.dt.float16

_NC = None
LAST_RESULT = None  # BassKernelResults of last run (test.py reads this)


def _build():
    nc = bacc.Bacc(None)
    qt_d = nc.declare_dram_parameter("qt_b", [N, DK], F32, isOutput=False)
    kt_d = nc.declare_dram_parameter("kt_b", [NK, DK], F32, isOutput=False)
    vt_d = nc.declare_dram_parameter("vt_b", [NK, DK], F32, isOutput=False)
    pr_d = nc.declare_dram_parameter("pr_b", [1, M], F32, isOutput=False)
    out_d = nc.declare_dram_parameter("out_b", [N, DK], F32, isOutput=True)

    with ExitStack() as ctx:
        tc = ctx.enter_context(tile.TileContext(nc))
        const = ctx.enter_context(tc.tile_pool(name="const", bufs=1))
        sbig = ctx.enter_context(tc.tile_pool(name="sbig", bufs=1))
        epool = ctx.enter_context(tc.tile_pool(name="epool", bufs=5))
        eppool = ctx.enter_context(tc.tile_pool(name="eppool", bufs=2))
        dpool = ctx.enter_context(tc.tile_pool(name="dpool", bufs=2))
        npool = ctx.enter_context(tc.tile_pool(name="npool", bufs=2))
        opool = ctx.enter_context(tc.tile_pool(name="opool", bufs=1))
        ps_s = ctx.enter_context(tc.tile_pool(name="ps_s", bufs=3, space="PSUM"))
        ps_acc = ctx.enter_context(tc.tile_pool(name="ps_acc", bufs=1, space="PSUM"))

        # ---- constants ----
        ident_f = const.tile([128, 128], F32)
        make_identity(nc, ident_f)
        ones_f = const.tile([128, 128], F32)
        nc.vector.memset(ones_f, 1.0)
        pr_sb = const.tile([128, M], F32)
        nc.sync.dma_start(
            out=pr_sb,
            in_=bass.AP(tensor=pr_d, offset=0, ap=[[0, 128], [1, M]]),
        )
        # 1/p_m broadcast on all partitions, then ones/p_m weight tiles (fp16)
        pr_rec = const.tile([128, M], F32)
        nc.vector.reciprocal(pr_rec, pr_sb)
        ones_p = []
        for m in range(M):
            t = const.tile([128, 128], F16, tag=f"ones_p{m}")
            nc.vector.tensor_scalar_mul(t, ones_f, pr_rec[:, m:m + 1])
            ones_p.append(t)

        # ---- PE warm-up: ~40 dependency-free matmuls back-to-back while the
        # input DMAs run. HAM un-throttles the PE clock (1.2->2.4 GHz) only
        # after ~3.4us of *sustained* PE activity; the main loop's bursts are
        # too short to ever trigger it on their own.
        warm_w = const.tile([128, 128], F16, tag="warm_w")
        nc.vector.tensor_copy(warm_w, ones_f)
        warm_r = const.tile([128, 512], F16, tag="warm_r")
        nc.vector.memset(warm_r, 0.5)
        warm_ps = ps_acc.tile([128, QHN], F32, tag="outT")
        for _ in range(40):
            nc.tensor.matmul(warm_ps[:, 0:512], lhsT=warm_w, rhs=warm_r,
                             start=True, stop=True)

        # ---- input staging: [128, 16, (m,d)] so stage[:, c, :] is a [128, 128]
        # block whose transpose has mixture m's d-rows at partitions m*64..m*64+63.
        # stage[p, c, m*64+d] = flat[m*131072 + (c*128+p)*64 + d]
        stages = []
        for src in (qt_d, kt_d):
            t = sbig.tile([128, NCH, DK], F32, tag=f"stage{len(stages)}")
            for m in range(M):
                for g in range(4):
                    nc.sync.dma_start(
                        out=t[:, g * 4:(g + 1) * 4, m * D:(m + 1) * D],
                        in_=bass.AP(
                            tensor=src, offset=m * N * D + g * 4 * 128 * D,
                            ap=[[D, 128], [128 * D, 4], [1, D]],
                        ),
                    )
            stages.append(t)

        # V: [128, 16, 128]  (p, c, dv) <- vt[c*128+p, dv]; convert to fp16
        v_st = sbig.tile([128, NCH, DV], F32)
        nc.sync.dma_start(
            out=v_st,
            in_=bass.AP(tensor=vt_d, offset=0,
                        ap=[[DK, 128], [128 * DK, NCH], [1, DV]]),
        )
        v_sb = sbig.tile([128, NCH, DV], F16)
        nc.vector.tensor_copy(v_sb, v_st)

        # ---- phase 1: QT/KT [128, 2048] fp16 (rows m*64+d), via PE transpose ----
        qt_t = sbig.tile([128, N], F16)
        kt_t = sbig.tile([128, NK], F16)
        for stage, dst in ((stages[0], qt_t), (stages[1], kt_t)):
            for g in range(4):
                tp = ps_s.tile([128, QHN], F32, tag="s")
                for j in range(4):
                    nc.tensor.transpose(tp[:, j * 128:(j + 1) * 128],
                                        stage[:, g * 4 + j, :], ident_f)
                nc.vector.tensor_copy(dst[:, g * 512:(g + 1) * 512], tp[:, 0:512])

        # ---- phase 2+3: attention ----
        # Normalization of segment i is deferred into segment i+1's chunk loop
        # (tree-adds early, Drep matmul late) so the PE queue never blocks on
        # DVE work at a boundary: PE goes straight from AV(15) to next S(0),
        # which also keeps the HAM clock-gate warm.
        scale = 1.0 / TEMP
        stage1 = {}  # c -> closure, emitted during the chunk loop
        stage2 = []  # closures, emitted after the full sweep

        def emit_stage1(c):
            for fn in stage1.pop(c, ()):  # noqa: B909
                fn()

        otn = {}
        daccs = {}
        ocs = {}
        drecs = {}
        rAs = {}
        segs = [(qh, m) for qh in range(QH) for m in range(M)]
        for qh, m in segs:
            is_last = (qh, m) == segs[-1]
            outT = ps_acc.tile([128, QHN], F32, tag="outT")
            ep = eppool.tile([128, NCH // 2, QHN], F16, tag="ep")
            es = []
            racc = None
            for c in range(NCH):
                s = ps_s.tile([128, QHN], F32, tag="s")
                for hf in range(2):
                    nc.tensor.matmul(
                        s[:, hf * 512:(hf + 1) * 512],
                        lhsT=kt_t[m * D:(m + 1) * D, c * 128:(c + 1) * 128],
                        rhs=qt_t[m * D:(m + 1) * D,
                                 qh * QHN + hf * 512: qh * QHN + (hf + 1) * 512],
                        start=True, stop=True,
                    )
                # deferred AV: consume E(c-2); exp(c-2) finished a chunk ago,
                # so the PE never waits on the ScalarE here
                if c >= 2:
                    Ep = es[c - 2]
                    for hf in range(2):
                        sl = slice(hf * 512, (hf + 1) * 512)
                        nc.tensor.matmul(outT[:, sl], lhsT=v_sb[:, c - 2, :],
                                         rhs=Ep[:, sl],
                                         start=(c == 2), stop=False)
                E = epool.tile([128, QHN], F16, tag="E")
                nc.scalar.activation(E, s, mybir.ActivationFunctionType.Exp,
                                     scale=scale)
                es.append(E)
                if c % 2 == 1:
                    nc.vector.tensor_add(ep[:, c // 2, :], es[c - 1], es[c])
                    # last segment: keep a running pair-sum so Dacc is ready
                    # ~one DVE op after the final exp (no post-loop tree tail)
                    if is_last and c >= 3:
                        r = dpool.tile([128, QHN], F16, tag="racc")
                        if c == 3:
                            nc.vector.tensor_add(r, ep[:, 0, :], ep[:, 1, :])
                        else:
                            nc.vector.tensor_add(r, racc, ep[:, c // 2, :])
                        racc = r
                emit_stage1(c)
            # tail AV for chunks 14, 15
            for ct in (NCH - 2, NCH - 1):
                for hf in range(2):
                    sl = slice(hf * 512, (hf + 1) * 512)
                    nc.tensor.matmul(outT[:, sl], lhsT=v_sb[:, ct, :],
                                     rhs=es[ct][:, sl],
                                     start=False, stop=(ct == NCH - 1))
            # free the single outT PSUM buffer right away: plain copy to SBUF;
            # the division by D happens later in SBUF during finish()
            oc = opool.tile([128, QHN], F32, tag=f"oc{qh}{m}")
            nc.vector.tensor_copy(oc, outT)
            ocs[(qh, m)] = oc

            def tree(qh=qh, m=m, ep=ep):
                # denominator tree: 8 pairs -> 4 -> 2 -> 1 (fp16), on DVE
                t4 = dpool.tile([128, 4, QHN], F16, tag="t4")
                nc.vector.tensor_add(t4, ep[:, 0:4, :], ep[:, 4:8, :])
                t2 = dpool.tile([128, 2, QHN], F16, tag="t2")
                nc.vector.tensor_add(t2, t4[:, 0:2, :], t4[:, 2:4, :])
                dacc = dpool.tile([128, QHN], F16, tag=f"dacc{m}")
                nc.vector.tensor_add(dacc, t2[:, 0, :], t2[:, 1, :])
                daccs[(qh, m)] = dacc

            def norm(qh=qh, m=m):
                # replicate D/p_m across partitions with a ones/p_m matmul
                dacc = daccs.pop((qh, m))
                Drep = ps_s.tile([128, QHN], F32, tag="s")
                for hf in range(2):
                    sl = slice(hf * 512, (hf + 1) * 512)
                    nc.tensor.matmul(Drep[:, sl], lhsT=ones_p[m], rhs=dacc[:, sl],
                                     start=True, stop=True)
                drec = opool.tile([128, QHN], F32, tag=f"drec{qh}{m}")
                nc.vector.reciprocal_approx_fast(drec, Drep)
                drecs[(qh, m)] = drec

            def finish(qh=qh):
                rA = npool.tile([128, QHN], F32, tag="rA")
                nc.vector.tensor_mul(rA, ocs[(qh, 0)], drecs[(qh, 0)])
                rB = npool.tile([128, QHN], F32, tag="rB")
                nc.vector.tensor_mul(rB, ocs[(qh, 1)], drecs[(qh, 1)])
                rT2 = npool.tile([128, QHN], F32, tag="rT2")
                nc.vector.tensor_add(rT2, rA, rB)
                res_ps = ps_s.tile([128, QHN], F32, tag="s")
                for t in range(QHN // 128):
                    nc.tensor.transpose(res_ps[:, t * 128:(t + 1) * 128],
                                        rT2[:, t * 128:(t + 1) * 128], ident_f)
                res_sb = npool.tile([128, QHN], F32, tag="res")
                nc.vector.tensor_copy(res_sb, res_ps)
                nc.sync.dma_start(
                    out=bass.AP(tensor=out_d, offset=qh * QHN * DK,
                                ap=[[DK, 128], [128 * DK, QHN // 128], [1, DV]]),
                    in_=res_sb.rearrange("p (t d) -> p t d", d=DV),
                )

            def finishA(qh=qh):
                rA = npool.tile([128, QHN], F32, tag="rA")
                nc.vector.tensor_mul(rA, ocs[(qh, 0)], drecs[(qh, 0)])
                rAs[qh] = rA

            def finishB(qh=qh):
                rB = npool.tile([128, QHN], F32, tag="rB")
                nc.vector.tensor_mul(rB, ocs[(qh, 1)], drecs[(qh, 1)])
                rT2 = npool.tile([128, QHN], F32, tag="rT2")
                nc.vector.tensor_add(rT2, rAs[qh], rB)
                res_ps = ps_s.tile([128, QHN], F32, tag="s")
                for t in range(QHN // 128):
                    nc.tensor.transpose(res_ps[:, t * 128:(t + 1) * 128],
                                        rT2[:, t * 128:(t + 1) * 128], ident_f)
                res_sb = npool.tile([128, QHN], F32, tag="res")
                nc.vector.tensor_copy(res_sb, res_ps)
                nc.sync.dma_start(
                    out=bass.AP(tensor=out_d, offset=qh * QHN * DK,
                                ap=[[DK, 128], [128 * DK, QHN // 128], [1, DV]]),
                    in_=res_sb.rearrange("p (t d) -> p t d", d=DV),
                )

            if not is_last:
                stage1.setdefault(2, []).append(tree)
                stage1.setdefault(8, []).append(norm)
                if m == M - 1:
                    stage1.setdefault(12, []).append(finish)
                if (qh, m) == segs[-2]:
                    # rA of the final qh can be computed as soon as its m=0
                    # drec lands, during the last segment's loop
                    stage1.setdefault(10, []).append(finishA)
            else:
                daccs[(qh, m)] = racc
                norm()
                finishB()
        stage1.clear()
    return nc


def _get_nc():
    global _NC
    if _NC is None:
        _NC = _build()
        _NC.finalize()  # Bacc.compile(): event sems, reg alloc, wait legalization
    return _NC


def _prior(qt, kernel):
    bar_qt = qt.astype(np.float32).mean(axis=1)          # (BS, dk)
    logits = kernel.astype(np.float32) @ bar_qt.T        # (m, BS)
    z = logits - logits.max(axis=1, keepdims=True)
    ez = np.exp(z)
    pm = ez / ez.sum(axis=1, keepdims=True)              # softmax over batch axis
    return pm.reshape(-1)


def kernel(qt, kt, vt, kernel):
    global LAST_RESULT
    import os
    nc = _get_nc()
    prior_flat = _prior(qt, kernel)
    in_maps = []
    for b in range(BS):
        pr = np.array([[prior_flat[2 * b], prior_flat[2 * b + 1]]], dtype=np.float32)
        in_maps.append({
            "qt_b": np.ascontiguousarray(qt[b], dtype=np.float32),
            "kt_b": np.ascontiguousarray(kt[b], dtype=np.float32),
            "vt_b": np.ascontiguousarray(vt[b], dtype=np.float32),
            "pr_b": pr,
        })
    trace = bool(int(os.environ.get("KERNEL_TRACE", "0")))
    res = run_bass_kernel_spmd(nc, in_maps, list(range(BS)), trace=trace)
    LAST_RESULT = res
    out = np.stack([np.asarray(res.results[b]["out_b"]).reshape(N, DK) for b in range(BS)])
    return out.astype(np.float32)
